# revision 5
# baseline (speedup 1.0000x reference)
"""Causal MHA on 8 TRN2 cores, single NEFF: TP-2 (head groups) x DP-4 (batch).

Core i = (batch i%4, head-group g=i//4). Uniform SPMD program; all per-core
asymmetry comes from input data (weight slices) and the pairwise
ReduceScatter (rank position inside the replica group).

Per core (g's 8 heads, one batch, full T=2048):
  xT   [c=1024, t=2048]      x_b transposed (host-prepped, bf16)
  Q^T  [512, 2048]           = Wq_g.T @ xT (+bq_g)     (4 j-tiles of 128)
  K^T  [512, 2048]           = Wk_g.T @ xT (+bk_g)
  V    [k, 16, 8, 65]        = xT.T @ Wv_g (+bv_g), col 64 = 1.0 (sum trick)
  per (qb in 0..3, head-pair p in 0..3), kmax = 512*(qb+1):
    S^T [k-tile 128, q 512]  = K^T.T @ Q^T (PSUM, 2 heads via row groups)
    P^T = exp(0.125 * S^T) * causal-mask                (bf16)
    A^T[65, q] += V.T @ P^T  (row 64 = softmax denominators)
    a[d', q] = A/denom                                  (bf16)
  y_part [2048, 1024] = a.T @ Wo_g (+bo/2)  -- partial over my 512 d'

Out_proj is row-parallel (Wo row-sharded across the pair), so each pair's
outputs are sum-sharded: y[b] = y_part(b, g0) + y_part(b, g1). Default
(USE_CC=False): each core returns its partial and kernel() unshards by
summing the pair on host (8M flops, 0.006% of the total). USE_CC=True
instead reduces on device with two pairwise ReduceScatters -- correct, but
measured ~210us slower (2-rank collective firmware latency, unoverlapped).
"""
import sys
sys.path.insert(0, '/opt/trn_rl_repo')
from contextlib import ExitStack

import numpy as np
import ml_dtypes

import concourse.bass as bass
import concourse.tile as tile
from concourse import bacc, mybir

BF16 = mybir.dt.bfloat16
F32 = mybir.dt.float32
AF = mybir.ActivationFunctionType
ALU = mybir.AluOpType

D = 1024
H = 16
HD = 64
T = 2048
B = 4
GW = 512           # per-group width (8 heads x 64)
SCALE = 1.0 / np.sqrt(HD)

# replica groups: pair (batch b) = cores (b, b+4)
RG = [[0, 4], [1, 5], [2, 6], [3, 7]]

# USE_CC: True -> on-device pairwise ReduceScatter; False -> each core
# outputs its row-parallel partial y [2048, D] (out_proj is row-sharded, so
# the output is sum-sharded across the pair) and kernel() unshards by
# summing the pair on the host. Measured on HW: the 2-rank ReduceScatter
# costs ~210us of NEFF execution time (firmware latency floor, not
# overlapped with compute), so the host-unshard variant is the default.
USE_CC = False


def build_attn8(num_devices: int = 8, use_cc: bool = USE_CC,
                fake_cc: bool = False):
    """fake_cc: replace the collective with a local DMA (single-core sim
    timing mode; output equals the local partial for shard rows)."""
    nc = bacc.Bacc("TRN2", target_bir_lowering=False, debug=False,
                   num_devices=num_devices)

    xT = nc.dram_tensor("xT", [D, T], BF16, kind="ExternalInput").ap()
    wq = nc.dram_tensor("wq", [D, GW], BF16, kind="ExternalInput").ap()
    wk = nc.dram_tensor("wk", [D, GW], BF16, kind="ExternalInput").ap()
    wv = nc.dram_tensor("wv", [D, GW], BF16, kind="ExternalInput").ap()
    wo = nc.dram_tensor("wo", [GW, D], BF16, kind="ExternalInput").ap()
    bqk = nc.dram_tensor("bqk", [128, 8], F32, kind="ExternalInput").ap()
    bvo = nc.dram_tensor("bvo", [2, D], BF16, kind="ExternalInput").ap()
    if use_cc:
        y = nc.dram_tensor("y", [2 * GW, D], BF16, kind="ExternalOutput").ap()
    else:
        y = nc.dram_tensor("y", [T, D], BF16, kind="ExternalOutput").ap()

    with tile.TileContext(nc) as tc, ExitStack() as ctx:
        nc = tc.nc
        consts = ctx.enter_context(tc.tile_pool(name="consts", bufs=1))
        big = ctx.enter_context(tc.tile_pool(name="big", bufs=1))
        wpool = ctx.enter_context(tc.tile_pool(name="w", bufs=1))
        ppool = ctx.enter_context(tc.tile_pool(name="p", bufs=3))
        rpool = ctx.enter_context(tc.tile_pool(name="r", bufs=2))
        ypool = ctx.enter_context(tc.tile_pool(name="y", bufs=4))
        ps = ctx.enter_context(tc.tile_pool(name="ps", bufs=2, space="PSUM"))
        dram = ctx.enter_context(tc.tile_pool(name="dram", bufs=1,
                                              space="DRAM"))

        # ---- constants ----
        # multiplicative causal mask: 1 where q(free) >= k(part), else 0
        mask2 = consts.tile([128, 2, 128], BF16, tag="mask2")
        nc.vector.memset(mask2[:], 1.0)
        nc.gpsimd.affine_select(
            out=mask2[:], in_=mask2[:], compare_op=ALU.is_ge, fill=0.0,
            base=0, pattern=[[0, 2], [1, 128]], channel_multiplier=-1)
        bq_sb = consts.tile([128, 4], F32, tag="bq")
        nc.sync.dma_start(bq_sb[:], bqk[:, 0:4])
        bk_sb = consts.tile([128, 4], F32, tag="bk")
        nc.sync.dma_start(bk_sb[:], bqk[:, 4:8])
        # partition-broadcast bv (cols 0:512) / bo_half to [128, *];
        # loaded on the gpsimd queue so they don't head-of-line-block the
        # wq/xT stream that gates the first matmuls
        bv_bc = consts.tile([128, GW], BF16, tag="bv")
        bo_bc = consts.tile([128, D], BF16, tag="bo")
        src = bass.AP(tensor=bvo.tensor, offset=0, ap=[[0, 128], [1, GW]])
        nc.gpsimd.dma_start(bv_bc[:], src)
        src = bass.AP(tensor=bvo.tensor, offset=D, ap=[[0, 128], [1, D]])
        nc.gpsimd.dma_start(bo_bc[:], src)

        # ---- big SBUF tensors ----
        xT_sb = big.tile([128, 8, T], BF16, tag="xT")
        qT_sb = big.tile([128, 4, T], BF16, tag="qT")
        kT_sb = big.tile([128, 4, T], BF16, tag="kT")
        v_sb = big.tile([128, 16, 8, 65], BF16, tag="v")
        a_sb = big.tile([128, 4, T], BF16, tag="a")
        nc.vector.memset(v_sb[:, :, :, 64:65], 1.0)

        xTr = xT.rearrange("(j p) k -> p j k", p=128)
        w_sb = {}

        def load_w(name, w, eng=None):
            t = wpool.tile([128, 8, GW], BF16, tag=name)
            wr = w.rearrange("(j p) d -> p j d", p=128)
            for j in range(8):
                (eng or nc.sync).dma_start(t[:, j, :], wr[:, j, :])
            w_sb[name] = t

        def load_xt(kb):
            for j in range(8):
                nc.sync.dma_start(
                    xT_sb[:, j, kb * 512:(kb + 1) * 512],
                    xTr[:, j, kb * 512:(kb + 1) * 512])

        # wq + xT[kb0] interleaved per c-tile so the first Q-proj group can
        # start after ~2 c-tiles instead of the full 2 MiB
        wq_t = wpool.tile([128, 8, GW], BF16, tag="wq")
        wqr = wq.rearrange("(j p) d -> p j d", p=128)
        for c in range(8):
            nc.sync.dma_start(wq_t[:, c, :], wqr[:, c, :])
            nc.sync.dma_start(xT_sb[:, c, 0:512], xTr[:, c, 0:512])
        w_sb["wq"] = wq_t
        load_w("wk", wk)
        for kb in range(1, 4):
            load_xt(kb)
        load_w("wv", wv, nc.scalar)
        # wo on the scalar queue; [512, D] -> [128, 4, D]
        wo_sb = wpool.tile([128, 4, D], BF16, tag="wo")
        wor = wo.rearrange("(j p) d -> p j d", p=128)
        for j in range(4):
            nc.scalar.dma_start(wo_sb[:, j, :], wor[:, j, :])

        def emit_qproj(qb, j, evac_dve=False):
            pt = ps.tile([128, 512], F32, tag="proj", bufs=2)
            for c in range(8):
                nc.tensor.matmul(
                    pt[:], w_sb["wq"][:, c, j * 128:(j + 1) * 128],
                    xT_sb[:, c, qb * 512:(qb + 1) * 512],
                    start=(c == 0), stop=(c == 7))
            if evac_dve:
                # once attention is running, ACT is saturated with exps --
                # evacuate on DVE so the psum slot frees promptly
                nc.vector.tensor_scalar_add(
                    qT_sb[:, j, qb * 512:(qb + 1) * 512], pt[:],
                    bq_sb[:, j:j + 1])
            else:
                nc.scalar.activation(
                    out=qT_sb[:, j, qb * 512:(qb + 1) * 512], in_=pt[:],
                    func=AF.Identity, bias=bq_sb[:, j:j + 1])

        def emit_kproj(kb, j, evac_dve=False):
            pt = ps.tile([128, 512], F32, tag="proj", bufs=2)
            for c in range(8):
                nc.tensor.matmul(
                    pt[:], w_sb["wk"][:, c, j * 128:(j + 1) * 128],
                    xT_sb[:, c, kb * 512:(kb + 1) * 512],
                    start=(c == 0), stop=(c == 7))
            if evac_dve:
                nc.vector.tensor_scalar_add(
                    kT_sb[:, j, kb * 512:(kb + 1) * 512], pt[:],
                    bk_sb[:, j:j + 1])
            else:
                nc.scalar.activation(
                    out=kT_sb[:, j, kb * 512:(kb + 1) * 512], in_=pt[:],
                    func=AF.Identity, bias=bk_sb[:, j:j + 1])

        def emit_vproj(kt):
            pt = ps.tile([128, 512], F32, tag="proj", bufs=2)
            for c in range(8):
                nc.tensor.matmul(
                    pt[:], xT_sb[:, c, kt * 128:(kt + 1) * 128],
                    w_sb["wv"][:, c, :],
                    start=(c == 0), stop=(c == 7))
            nc.vector.tensor_tensor(
                out=v_sb[:, kt, :, 0:64],
                in0=pt[:].rearrange("p (h d) -> p h d", d=64),
                in1=bv_bc[:].rearrange("p (h d) -> p h d", d=64),
                op=ALU.add)

        def emit_attn(qb, p):
            q0 = qb * 512
            nkt = 4 * (qb + 1)
            apsA = ps.tile([128, 512], F32, tag="acc", bufs=2)
            apsB = ps.tile([128, 512], F32, tag="acc", bufs=2)
            for kt in range(nkt):
                qoff = max(0, 128 * kt - q0)
                spair = ps.tile([128, 2, 512], F32, tag="s", bufs=2)
                for hh in range(2):
                    pr = slice(hh * 64, hh * 64 + 64)
                    nc.tensor.matmul(
                        spair[:, hh, qoff:512],
                        kT_sb[pr, p, kt * 128:(kt + 1) * 128],
                        qT_sb[pr, p, q0 + qoff:q0 + 512],
                        start=True, stop=True)
                diag = (128 * kt >= q0)
                ppair = ppool.tile([128, 2, 512], BF16, tag="ppair")
                nc.scalar.activation(
                    out=ppair[:, :, qoff:512], in_=spair[:, :, qoff:512],
                    func=AF.Exp, scale=SCALE)
                if diag:
                    nc.vector.tensor_tensor(
                        out=ppair[:, :, qoff:qoff + 128],
                        in0=ppair[:, :, qoff:qoff + 128],
                        in1=mask2[:], op=ALU.mult)
                for hh, aps in ((0, apsA), (1, apsB)):
                    nc.tensor.matmul(
                        aps[0:65, qoff:512], v_sb[:, kt, 2 * p + hh, :],
                        ppair[:, hh, qoff:512],
                        start=(kt == 0), stop=(kt == nkt - 1))
            last = (qb == 3 and p == 3)
            for hh, aps in ((0, apsA), (1, apsB)):
                acop = rpool.tile([65, 512], F32, tag="acop")
                if last:
                    # tail: ACT is idle once the final exps retire; keep the
                    # normalize chain off the backed-up DVE queue
                    nc.scalar.copy(acop[:], aps[0:65, :])
                else:
                    nc.vector.tensor_copy(acop[:], aps[0:65, :])
                recip = rpool.tile([1, 512], F32, tag="recip")
                nc.vector.reciprocal(recip[:], acop[64:65, :])
                bc_sb = rpool.tile([64, 512], F32, tag="bc_sb")
                nc.gpsimd.partition_broadcast(bc_sb[:], recip[:])
                if hh == 0:
                    nc.vector.tensor_tensor(
                        out=a_sb[0:64, p, q0:q0 + 512],
                        in0=acop[0:64, :], in1=bc_sb[:], op=ALU.mult)
                else:
                    stage = rpool.tile([64, 512], BF16, tag="stage")
                    nc.vector.tensor_tensor(
                        out=stage[:], in0=acop[0:64, :], in1=bc_sb[:],
                        op=ALU.mult)
                    nc.gpsimd.dma_start(
                        a_sb[64:128, p, q0:q0 + 512], stage[:])

        # partial-y staging in DRAM for the collectives:
        # ccA rows {[0:512) , [1024:1536)}  = qb0, qb2
        # ccB rows {[512:1024), [1536:2048)} = qb1, qb3
        if use_cc:
            ccA_in = dram.tile([2, 512, D], BF16, tag="ccA_in")
            ccB_in = dram.tile([2, 512, D], BF16, tag="ccB_in")
            ccA_out = dram.tile([512, D], BF16, tag="ccA_out")
            ccB_out = dram.tile([512, D], BF16, tag="ccB_out")

        def emit_opartial(qt):
            # y_part[q, e] for one 128-row query tile; contraction over my
            # 512 d' (4 p-tiles)
            yt = ypool.tile([128, D], BF16, tag="y")
            for n in range(2):
                pt = ps.tile([128, 512], F32, tag="proj", bufs=2)
                for p in range(4):
                    nc.tensor.matmul(
                        pt[:], a_sb[:, p, qt * 128:(qt + 1) * 128],
                        wo_sb[:, p, n * 512:(n + 1) * 512],
                        start=(p == 0), stop=(p == 3))
                nc.vector.tensor_tensor(
                    out=yt[:, n * 512:(n + 1) * 512], in0=pt[:],
                    in1=bo_bc[:, n * 512:(n + 1) * 512], op=ALU.add)
            if not use_cc:
                nc.sync.dma_start(y[qt * 128:(qt + 1) * 128, :], yt[:])
                return
            qb = qt // 4
            sh = qb // 2            # half: qb0,1 -> shard 0; qb2,3 -> shard 1
            cc = ccA_in if qb % 2 == 0 else ccB_in
            r0 = (qt % 4) * 128
            # gpsimd queue: keep these writes off the sync queue (input
            # loads) and away from DVE (attention mask/normalize ops)
            nc.gpsimd.dma_start(cc[sh, r0:r0 + 128, :], yt[:])

        # ---- emission schedule ----
        # qb0 deps first, then per-qb: K/V proj -> attention -> partial O.
        for j in range(4):
            emit_qproj(0, j)
        # V before K: wv arrives on the scalar queue well before wk clears
        # the sync queue, and PE is strict FIFO — K MMs at the queue head
        # waiting on wk would block ready V MMs behind them
        for kt in range(4):
            emit_vproj(kt)
        for j in range(4):
            emit_kproj(0, j)
        for qb in range(1, 4):
            for j in range(4):
                emit_qproj(qb, j, evac_dve=True)
        for p in range(4):
            emit_attn(0, p)
        for qb in range(1, 4):
            for j in range(4):
                emit_kproj(qb, j, evac_dve=True)
            for kt in range(4 * qb, 4 * qb + 4):
                emit_vproj(kt)
            for qt in range(4 * (qb - 1), 4 * qb):
                emit_opartial(qt)
            with tc.high_priority():
                for p in range(4):
                    emit_attn(qb, p)
        for qt in range(12, 16):
            emit_opartial(qt)

        if use_cc:
            if fake_cc:
                nc.sync.dma_start(ccA_out[:, :], ccA_in[0, :, :])
                nc.sync.dma_start(ccB_out[:, :], ccB_in[0, :, :])
            else:
                nc.gpsimd.collective_compute(
                    "ReduceScatter", ALU.add, replica_groups=RG,
                    ins=[ccA_in.opt()], outs=[ccA_out.opt()])
                nc.gpsimd.collective_compute(
                    "ReduceScatter", ALU.add, replica_groups=RG,
                    ins=[ccB_in.opt()], outs=[ccB_out.opt()])
            # local y rows [0:512) = my half's first 512 rows (ccA),
            # [512:1024) = second 512 rows (ccB)
            nc.sync.dma_start(y[0:512, :], ccA_out[:, :])
            nc.sync.dma_start(y[512:1024, :], ccB_out[:, :])

    nc.compile()
    return nc


# ---------------- host-side helpers ----------------

def make_core_inputs8(x, Wq, bq, Wk, bk, Wv, bv, Wo, bo, b, g):
    bf = ml_dtypes.bfloat16
    xb = np.asarray(x[b], dtype=np.float32)     # [T, D]
    sl = slice(g * GW, (g + 1) * GW)
    bqg = np.asarray(bq, np.float32)[sl].reshape(4, 128).T
    bkg = np.asarray(bk, np.float32)[sl].reshape(4, 128).T
    bvo = np.zeros((2, D), np.float32)
    bvo[0, :GW] = np.asarray(bv, np.float32)[sl]
    bvo[1] = 0.5 * np.asarray(bo, np.float32)
    return {
        "xT": np.ascontiguousarray(xb.T).astype(bf),
        "wq": np.ascontiguousarray(np.asarray(Wq, np.float32)[:, sl]).astype(bf),
        "wk": np.ascontiguousarray(np.asarray(Wk, np.float32)[:, sl]).astype(bf),
        "wv": np.ascontiguousarray(np.asarray(Wv, np.float32)[:, sl]).astype(bf),
        "wo": np.ascontiguousarray(np.asarray(Wo, np.float32)[sl, :]).astype(bf),
        "bqk": np.concatenate([bqg, bkg], axis=1),
        "bvo": bvo.astype(bf),
    }


def assemble_output8(core_outs, use_cc=USE_CC):
    """core_outs: 8 per-core y arrays in device order -> [B, T, D] f32."""
    out = np.empty((B, T, D), np.float32)
    for core in range(8):
        b, g = core % 4, core // 4
        if use_cc:
            out[b, g * 1024:(g + 1) * 1024] = np.asarray(
                core_outs[core], np.float32)
        else:
            if g == 0:
                out[b] = np.asarray(core_outs[core], np.float32)
            else:
                out[b] += np.asarray(core_outs[core], np.float32)
    return out


# ======================= runner (host side) =======================
import jax
from jax.sharding import Mesh, PartitionSpec, NamedSharding
from jax.experimental.shard_map import shard_map
from concourse import bass2jax


def _make_fn(nc, devs):
    pname = nc.partition_id_tensor.name if nc.partition_id_tensor else None
    in_names, out_names, out_avals, zero_outs = [], [], [], []
    for alloc in nc.m.functions[0].allocations:
        if not isinstance(alloc, mybir.MemoryLocationSet):
            continue
        name = alloc.memorylocations[0].name
        if alloc.kind == "ExternalInput":
            if name != pname:
                in_names.append(name)
        elif alloc.kind == "ExternalOutput":
            out_names.append(name)
            shape = tuple(alloc.tensor_shape)
            dtype = mybir.dt.np(alloc.dtype)
            out_avals.append(jax.core.ShapedArray(shape, dtype))
            zero_outs.append(np.zeros(shape, dtype))
    n_params = len(in_names)
    all_names = in_names + out_names + ([pname] if pname else [])

    def _body(*args):
        args = list(args)
        if pname:
            args.append(bass2jax.partition_id_tensor())
        outs = bass2jax._bass_exec_p.bind(
            *args, out_avals=tuple(out_avals), in_names=tuple(all_names),
            out_names=tuple(out_names), lowering_input_output_aliases=(),
            sim_require_finite=False, sim_require_nnan=False, nc=nc)
        return tuple(outs)

    mesh = Mesh(np.asarray(devs), ("core",))
    nio = n_params + len(out_names)
    f = jax.jit(shard_map(_body, mesh=mesh,
                          in_specs=(PartitionSpec("core"),) * nio,
                          out_specs=(PartitionSpec("core"),) * len(out_names),
                          check_rep=False), keep_unused=True)
    return f, in_names, out_names, zero_outs, mesh


class _AttnRunner:
    """One 8-core NEFF; core i = (batch i%4, head-group i//4)."""

    def __init__(self):
        bass2jax.install_neuronx_cc_hook()
        devs = jax.devices()
        assert len(devs) >= 8, f"need 8 neuron cores, have {len(devs)}"
        nc = build_attn8(num_devices=8)
        f, inn, outn, zo, mesh = _make_fn(nc, devs[:8])
        self.parts = [dict(f=f, in_names=inn, out_names=outn, zero_outs=zo,
                           mesh=mesh)]

    def prepare(self, **inputs):
        part = self.parts[0]
        per_core = [make_core_inputs8(b=c % 4, g=c // 4, **inputs)
                    for c in range(8)]
        sh = NamedSharding(part["mesh"], PartitionSpec("core"))
        cin = [jax.device_put(
            np.concatenate([pc[k] for pc in per_core], axis=0), sh)
            for k in part["in_names"]]
        cz = [jax.device_put(
            np.zeros((8 * z.shape[0], *z.shape[1:]), z.dtype), sh)
            for z in part["zero_outs"]]
        staged = [(cin, cz)]
        jax.block_until_ready([s[0] for s in staged])
        return staged

    def dispatch(self, staged):
        return [part["f"](*cin, *cz)
                for part, (cin, cz) in zip(self.parts, staged)]

    def run(self, staged):
        outs = self.dispatch(staged)
        jax.block_until_ready(outs)
        rows = 2 * GW if USE_CC else T
        yv = np.asarray(outs[0][0]).reshape(8, rows, D)
        return assemble_output8([yv[c] for c in range(8)])


_RUNNER = None


def kernel(**inputs):
    """Full-input causal MHA on 8 NeuronCores; returns [B, T, D] float32."""
    global _RUNNER
    inputs = {k: np.asarray(v) for k, v in inputs.items()}
    if _RUNNER is None:
        _RUNNER = _AttnRunner()
    staged = _RUNNER.prepare(**inputs)
    return _RUNNER.run(staged)
